# revision 16
# baseline (speedup 1.0000x reference)
"""Trainium2 Bass kernel for the NeuralFVSolver problem.

Strategy: pure data parallel over batch (16 batches -> 8 cores x 2).
Per core, the 63 autoregressive steps run fully unrolled. Per step:
  - shock detection is_shock[j] = state[j] > state[j-1]  (the reference's
    char_L > s_rh > char_R condition algebraically reduces to rR > rL)
  - prox computed directly in exp space: prox[i] = max_j m[j]*alpha^|i-j|
    with alpha = exp(-dx/sigma), via max-mult tensor_tensor_scan passes
    (1D distance transform in the max-product semiring; far field
    underflows to 0 exactly like the reference's exp(-1e6/sigma))
  - the backward scan is split at cell 509 (AP-chained initial) so the
    high half's stencil DMA can issue early and its ~2.2us completion
    latency hides under the MLP ladder, which processes the high half
    first
  - shock pre-ops are split at the 513/511 boundary so they run in the
    shadow of the other half's MLP ladder
  - stencil features built with sliding-window DMAs into [14,1024] tiles
  - 4-layer MLP as block-diagonal float32r matmuls processing both batch
    rows in one pass; char_speed and dt channels folded into W0/biases
    host-side; exact-erf Gelu with fused bias on the scalar engine
  - state update (+b3) and clip fused into two DVE ops per half
"""

import math
import numpy as np
from contextlib import ExitStack

import concourse.bass as bass
import concourse.bacc as bacc
import concourse.tile as tile
from concourse import mybir
from concourse.bass_utils import run_bass_kernel_spmd
from concourse.tile_rust import add_dep_helper

F32 = mybir.dt.float32
F32R = mybir.dt.float32r
BF16 = mybir.dt.bfloat16
OP = mybir.AluOpType
AF = mybir.ActivationFunctionType

B, NT, NX = 16, 64, 1024
NSTEP = NT - 1
NCORES = 8
BPC = B // NCORES          # batches per core = 2
KHW = 3                    # stencil half width
S = 2 * KHW + 1            # 7
DX = 0.02
SIGMA = 0.05
HID = 64
PAD = NX + 2 * KHW         # 1030
CH = 512                   # matmul moving-dim chunk (fp32 PSUM bank limit)
SPL = CH - KHW             # 509: backward-scan split point

ALPHA = float(np.float32(math.exp(-DX / SIGMA)))       # per-cell decay
BASE0 = float(np.float32(math.exp(-0.5 * DX / SIGMA))) # half-cell seed

_compiled = None


def _build_module():
    nc = bacc.Bacc("TRN2", target_bir_lowering=False, debug=False)

    d_state0 = nc.dram_tensor("state0", [BPC, PAD], F32, kind="ExternalInput").ap()
    d_l0v = nc.dram_tensor("l0v", [2 * S, 2 * HID], F32, kind="ExternalInput").ap()
    d_l0p = nc.dram_tensor("l0p", [S, BPC, 2 * HID], F32, kind="ExternalInput").ap()
    d_l1 = nc.dram_tensor("l1", [2 * HID, 2 * HID], F32, kind="ExternalInput").ap()
    d_l2 = nc.dram_tensor("l2", [2 * HID, 2 * HID], F32, kind="ExternalInput").ap()
    d_l3 = nc.dram_tensor("l3", [2 * HID, BPC], F32, kind="ExternalInput").ap()
    d_b0 = nc.dram_tensor("b0d", [2 * HID, 1], F32, kind="ExternalInput").ap()
    d_b1 = nc.dram_tensor("b1d", [2 * HID, 1], F32, kind="ExternalInput").ap()
    d_b2 = nc.dram_tensor("b2d", [2 * HID, 1], F32, kind="ExternalInput").ap()
    d_b3 = nc.dram_tensor("b3d", [BPC, 1], F32, kind="ExternalInput").ap()
    d_out = nc.dram_tensor("out", [BPC, NSTEP, NX], F32, kind="ExternalOutput").ap()

    with tile.TileContext(nc) as tc, ExitStack() as ctx:
        pool = ctx.enter_context(tc.tile_pool(name="sb", bufs=1))
        psum = ctx.enter_context(tc.tile_pool(name="ps", bufs=1, space="PSUM"))

        p0 = pool.tile([BPC, PAD], F32, tag="p0")
        p1 = pool.tile([BPC, PAD], F32, tag="p1")
        q = pool.tile([BPC, PAD], F32R, tag="q")
        sh = pool.tile([BPC, NX + 1], BF16, tag="sh")
        m = pool.tile([BPC, NX], BF16, tag="m")
        bse = pool.tile([BPC, NX], F32, tag="bse")
        Pf = pool.tile([BPC, NX], F32, tag="Pf")
        alc = pool.tile([BPC, NX], F32, tag="alc")
        tmp = pool.tile([BPC, NX], F32, tag="tmp")
        rhs_v = pool.tile([2 * S, NX], F32R, tag="rhs_v")
        a0 = pool.tile([2 * HID, NX], F32R, tag="a0")
        a1 = pool.tile([2 * HID, NX], F32R, tag="a1")
        a2 = pool.tile([2 * HID, NX], F32R, tag="a2")

        l0v_f = pool.tile([2 * S, 2 * HID], F32, tag="l0v_f")
        l1_f = pool.tile([2 * HID, 2 * HID], F32, tag="l1_f")
        l2_f = pool.tile([2 * HID, 2 * HID], F32, tag="l2_f")
        l3_f = pool.tile([2 * HID, BPC], F32, tag="l3_f")
        l0v = pool.tile([2 * S, 2 * HID], F32R, tag="l0v")
        l0ps_f = [pool.tile([BPC, 2 * HID], F32, tag=f"l0p{s}_f",
                            name=f"l0p{s}_f") for s in range(S)]
        l0ps = [pool.tile([BPC, 2 * HID], F32R, tag=f"l0p{s}",
                          name=f"l0p{s}") for s in range(S)]
        junk_w = pool.tile([2 * S, 2 * HID], F32R, tag="junk_w")
        l1 = pool.tile([2 * HID, 2 * HID], F32R, tag="l1")
        l2 = pool.tile([2 * HID, 2 * HID], F32R, tag="l2")
        l3 = pool.tile([2 * HID, BPC], F32R, tag="l3")
        b0t = pool.tile([2 * HID, 1], F32, tag="b0t")
        b1t = pool.tile([2 * HID, 1], F32, tag="b1t")
        b2t = pool.tile([2 * HID, 1], F32, tag="b2t")
        b3t = pool.tile([BPC, 1], F32, tag="b3t")

        h0 = psum.tile([2 * HID, NX], F32, tag="h0")
        h1 = psum.tile([2 * HID, NX], F32, tag="h1")
        h2 = psum.tile([2 * HID, NX], F32, tag="h2")
        upd = psum.tile([BPC, NX], F32, tag="upd")

        # ---- one-time loads / inits ----
        nc.sync.dma_start(p0[:], d_state0[:])
        for t_, d_ in ((l0v_f, d_l0v), (l1_f, d_l1),
                       (l2_f, d_l2), (l3_f, d_l3),
                       (b0t, d_b0), (b1t, d_b1), (b2t, d_b2), (b3t, d_b3)):
            nc.sync.dma_start(t_[:], d_[:])
        for s in range(S):
            nc.sync.dma_start(l0ps_f[s][:], d_l0p[s])
        for src, dst in ((l0v_f, l0v), (l1_f, l1),
                         (l2_f, l2), (l3_f, l3), *zip(l0ps_f, l0ps)):
            nc.vector.tensor_copy(dst[:], src[:])
        nc.vector.memset(junk_w[:].bitcast(F32), 0.0)
        nc.vector.memset(alc[:], ALPHA)
        nc.vector.memset(sh[:, NX:NX + 1], 0.0)

        def sliding(ptile, c0, n):
            # [BPC, S, n] overlapping-window view starting at cell c0
            src = ptile[:, c0:c0 + S]
            fancy = src.copy()
            fancy.ap = src.ap + [[1, n]]
            return fancy.bitcast(F32R)

        def preops(p_cur, lo, hi):
            # sh[j] = state[j] > state[j-1] over [lo, hi); m/bse over [lo, hi)
            nc.vector.tensor_tensor(sh[:, lo:hi], p_cur[:, KHW + lo:KHW + hi],
                                    p_cur[:, KHW + lo - 1:KHW + hi - 1], OP.is_gt)
            nc.vector.tensor_tensor(m[:, lo:hi], sh[:, lo:hi],
                                    sh[:, lo + 1:hi + 1], OP.max)
            return nc.vector.tensor_scalar(bse[:, lo:hi], m[:, lo:hi],
                                           BASE0, None, OP.mult)

        def filler(rhs, dep):
            fi = nc.tensor.matmul(h2[:, 0:CH], junk_w[:], rhs,
                                  start=True, stop=True, skip_group_check=True)
            if dep is not None:
                add_dep_helper(fi.ins, dep.ins, reason="pe warmup throttle")
            return fi

        def prox_mms(c0):
            for s in range(S):
                nc.tensor.matmul(h0[:, c0:c0 + CH], l0ps[s][:],
                                 q[:, c0 + s:c0 + s + CH],
                                 start=False, stop=(s == S - 1))

        def ladder_half(c0, first):
            cs = slice(c0, c0 + CH)
            nc.scalar.activation(a0[:, cs], h0[:, cs], AF.Gelu, bias=b0t[:])
            nc.tensor.matmul(h1[:, cs], l1[:], a0[:, cs], start=True, stop=True)
            nc.scalar.activation(a1[:, cs], h1[:, cs], AF.Gelu, bias=b1t[:])
            nc.tensor.matmul(h2[:, cs], l2[:], a1[:, cs], start=True, stop=True)
            nc.scalar.activation(a2[:, cs], h2[:, cs], AF.Gelu, bias=b2t[:])
            nc.tensor.matmul(upd[:, cs], l3[:], a2[:, cs], start=True, stop=True)

        def finish_half(p_cur, c0):
            cs = slice(c0, c0 + CH)
            nc.vector.scalar_tensor_tensor(tmp[:, cs], upd[:, cs], b3t[:],
                                           p_prev[:, KHW + c0:KHW + c0 + CH],
                                           OP.add, OP.add)
            nc.vector.tensor_scalar(p_cur[:, KHW + c0:KHW + c0 + CH],
                                    tmp[:, cs], 0.0, 1.0, OP.max, OP.min)

        for t in range(NSTEP):
            p_prev = p0 if t % 2 == 0 else p1
            p_cur = p1 if t % 2 == 0 else p0

            # vals stencil DMA + early L0-vals matmuls
            nc.scalar.dma_start(rhs_v[:], sliding(p_prev, 0, NX))
            nc.tensor.matmul(h0[:, CH:NX], l0v[:], rhs_v[:, CH:NX],
                             start=True, stop=False)
            nc.tensor.matmul(h0[:, 0:CH], l0v[:], rhs_v[:, 0:CH],
                             start=True, stop=False)

            if t == 0:
                last_bse = None
                # edges + full shock pre-ops for the initial state
                nc.vector.tensor_copy(p_prev[:, 0:KHW],
                                      p_prev[:, KHW:KHW + 1].broadcast_to([BPC, KHW]))
                nc.vector.tensor_copy(p_prev[:, KHW + NX:PAD],
                                      p_prev[:, KHW + NX - 1:KHW + NX]
                                      .broadcast_to([BPC, KHW]))
                last_bse = preops(p_prev, 0, NX)

            # PE warm-up fillers, throttled via explicit scheduler deps
            filler(rhs_v[:, 0:CH], last_bse)
            # ---- distance transform in exp space (vector engine) ----
            pf_i = nc.vector.tensor_tensor_scan(Pf[:], alc[:], bse[:], 0.0,
                                                OP.mult, OP.max)
            filler(rhs_v[:, CH:NX], pf_i)
            # backward scan, high part first (cells SPL..NX-1)
            nc.vector.tensor_tensor_scan(
                q[:, KHW + SPL:KHW + NX][:, ::-1], alc[:, 0:NX - SPL],
                Pf[:, SPL:NX][:, ::-1], 0.0, OP.mult, OP.max)
            nc.vector.tensor_copy(q[:, KHW + NX:PAD],
                                  q[:, KHW + NX - 1:KHW + NX]
                                  .broadcast_to([BPC, KHW]))
            prox_mms(CH)
            # backward scan low part, chained via initial
            nc.vector.tensor_tensor_scan(
                q[:, KHW:KHW + SPL][:, ::-1], alc[:, 0:SPL],
                Pf[:, 0:SPL][:, ::-1], q[:, KHW + SPL:KHW + SPL + 1],
                OP.mult, OP.max)
            nc.vector.tensor_copy(q[:, 0:KHW],
                                  q[:, KHW:KHW + 1].broadcast_to([BPC, KHW]))
            prox_mms(0)

            # ---- MLP ladder: high half first, then low half ----
            ladder_half(CH, True)
            ladder_half(0, False)

            # high half finishes first; its shock pre-ops run in the low
            # ladder's shadow
            finish_half(p_cur, CH)
            nc.vector.tensor_copy(p_cur[:, KHW + NX:PAD],
                                  p_cur[:, KHW + NX - 1:KHW + NX]
                                  .broadcast_to([BPC, KHW]))
            preops(p_cur, CH + 1, NX)

            finish_half(p_cur, 0)
            nc.vector.tensor_copy(p_cur[:, 0:KHW],
                                  p_cur[:, KHW:KHW + 1].broadcast_to([BPC, KHW]))
            last_bse = preops(p_cur, 0, CH + 1)

            nc.scalar.dma_start(d_out[:, t, :], p_cur[:, KHW:KHW + NX])

    nc.compile()
    return nc


def _prepare_core_inputs(grid_input, dt, W0, b0, W1, b1, W2, b2, W3, b3):
    """Host-side constant folding; returns list of per-core input dicts."""
    f = np.float32
    W0 = W0.astype(f); W1 = W1.astype(f); W2 = W2.astype(f); W3 = W3.astype(f)
    W0v = W0[:, 0:S] - 2.0 * W0[:, S:2 * S]          # vals + folded char_speeds
    W0p = W0[:, 2 * S:3 * S]                          # prox columns

    def blockdiag(Wsub):  # Wsub [HID, k] -> [2k, 2HID]
        k = Wsub.shape[1]
        out = np.zeros((2 * k, 2 * HID), f)
        out[0:k, 0:HID] = Wsub.T
        out[k:2 * k, HID:2 * HID] = Wsub.T
        return out

    l0v = blockdiag(W0v)
    l0p = blockdiag(W0p)
    l1b = blockdiag(W1)
    l2b = blockdiag(W2)

    in_maps = []
    for c in range(NCORES):
        bsel = [BPC * c + i for i in range(BPC)]
        dts = dt[bsel].astype(f)
        s0 = grid_input[bsel, 0, 0, :].astype(f)
        s0p = np.concatenate([np.repeat(s0[:, :1], KHW, 1), s0,
                              np.repeat(s0[:, -1:], KHW, 1)], axis=1)
        b0d = np.concatenate([
            (b0 + W0[:, S:2 * S].sum(1) + W0[:, 3 * S] * dts[0]),
            (b0 + W0[:, S:2 * S].sum(1) + W0[:, 3 * S] * dts[1]),
        ]).astype(f)[:, None]
        b1d = np.concatenate([b1, b1]).astype(f)[:, None]
        b2d = np.concatenate([b2, b2]).astype(f)[:, None]
        l3 = np.zeros((2 * HID, BPC), f)
        l3[0:HID, 0] = W3[0] * (dts[0] / DX)
        l3[HID:2 * HID, 1] = W3[0] * (dts[1] / DX)
        b3d = np.array([[b3[0] * dts[0] / DX], [b3[0] * dts[1] / DX]], f)
        in_maps.append({
            "state0": s0p, "l0v": l0v, "l0p": l0p, "l1": l1b, "l2": l2b,
            "l3": l3, "b0d": b0d, "b1d": b1d, "b2d": b2d, "b3d": b3d,
        })
    return in_maps


def kernel(grid_input, dt, W0, b0, W1, b1, W2, b2, W3, b3, _run_kwargs=None):
    global _compiled
    grid_input = np.asarray(grid_input)
    if _compiled is None:
        _compiled = _build_module()
    nc = _compiled
    in_maps = _prepare_core_inputs(grid_input, np.asarray(dt),
                                   np.asarray(W0), np.asarray(b0),
                                   np.asarray(W1), np.asarray(b1),
                                   np.asarray(W2), np.asarray(b2),
                                   np.asarray(W3), np.asarray(b3))
    kw = _run_kwargs or {}
    r = run_bass_kernel_spmd(nc, in_maps, list(range(NCORES)), **kw)
    out = np.empty((B, 1, NT, NX), np.float32)
    out[:, 0, 0, :] = grid_input[:, 0, 0, :]
    for c in range(NCORES):
        out[BPC * c:BPC * (c + 1), 0, 1:, :] = r.results[c]["out"]
    kernel.last_results = r
    return out


# revision 21
# speedup vs baseline: 1.0575x; 1.0575x over previous
"""Trainium2 Bass kernel for the NeuralFVSolver problem.

Strategy: pure data parallel over batch (16 batches -> 8 cores x 2).
Per core, the 63 autoregressive steps run fully unrolled. Per step:
  - shock detection is_shock[j] = state[j] > state[j-1]  (the reference's
    char_L > s_rh > char_R condition algebraically reduces to rR > rL)
  - prox computed directly in exp space: prox[i] = max_j m[j]*alpha^|i-j|
    with alpha = exp(-dx/sigma), via max-mult tensor_tensor_scan passes
    (1D distance transform in the max-product semiring; far field
    underflows to 0 exactly like the reference's exp(-1e6/sigma))
  - the backward scan is split at cell 509 (AP-chained initial) so the
    high half's stencil DMA can issue early and its ~2.2us completion
    latency hides under the MLP ladder, which processes the high half
    first
  - shock pre-ops are split at the 513/511 boundary so they run in the
    shadow of the other half's MLP ladder
  - stencil features built with sliding-window DMAs into [14,1024] tiles
  - 4-layer MLP as block-diagonal float32r matmuls processing both batch
    rows in one pass; char_speed and dt channels folded into W0/biases
    host-side; exact-erf Gelu with fused bias on the scalar engine
  - state update (+b3) and clip fused into two DVE ops per half
"""

import math
import numpy as np
from contextlib import ExitStack

import concourse.bass as bass
import concourse.bacc as bacc
import concourse.tile as tile
from concourse import mybir
from concourse.bass_utils import run_bass_kernel_spmd

F32 = mybir.dt.float32
F32R = mybir.dt.float32r
BF16 = mybir.dt.bfloat16
OP = mybir.AluOpType
AF = mybir.ActivationFunctionType

B, NT, NX = 16, 64, 1024
NSTEP = NT - 1
NCORES = 8
BPC = B // NCORES          # batches per core = 2
KHW = 3                    # stencil half width
S = 2 * KHW + 1            # 7
DX = 0.02
SIGMA = 0.05
HID = 64
PAD = NX + 2 * KHW         # 1030
CH = 512                   # matmul moving-dim chunk (fp32 PSUM bank limit)
SPL = CH - KHW             # 509: backward-scan split point

ALPHA = float(np.float32(math.exp(-DX / SIGMA)))       # per-cell decay
BASE0 = float(np.float32(math.exp(-0.5 * DX / SIGMA))) # half-cell seed

_compiled = None


def _build_module():
    nc = bacc.Bacc("TRN2", target_bir_lowering=False, debug=False)

    d_state0 = nc.dram_tensor("state0", [BPC, PAD], F32, kind="ExternalInput").ap()
    d_l0v = nc.dram_tensor("l0v", [2 * S, 2 * HID], F32, kind="ExternalInput").ap()
    d_l0p = nc.dram_tensor("l0p", [2 * S, 2 * HID], F32, kind="ExternalInput").ap()
    d_l1 = nc.dram_tensor("l1", [2 * HID, 2 * HID], F32, kind="ExternalInput").ap()
    d_l2 = nc.dram_tensor("l2", [2 * HID, 2 * HID], F32, kind="ExternalInput").ap()
    d_l3 = nc.dram_tensor("l3", [2 * HID, BPC], F32, kind="ExternalInput").ap()
    d_b0 = nc.dram_tensor("b0d", [2 * HID, 1], F32, kind="ExternalInput").ap()
    d_b1 = nc.dram_tensor("b1d", [2 * HID, 1], F32, kind="ExternalInput").ap()
    d_b2 = nc.dram_tensor("b2d", [2 * HID, 1], F32, kind="ExternalInput").ap()
    d_b3 = nc.dram_tensor("b3d", [BPC, 1], F32, kind="ExternalInput").ap()
    d_out = nc.dram_tensor("out", [BPC, NSTEP, NX], F32, kind="ExternalOutput").ap()

    with tile.TileContext(nc) as tc, ExitStack() as ctx:
        pool = ctx.enter_context(tc.tile_pool(name="sb", bufs=1))
        psum = ctx.enter_context(tc.tile_pool(name="ps", bufs=1, space="PSUM"))

        p0 = pool.tile([BPC, PAD], F32, tag="p0")
        p1 = pool.tile([BPC, PAD], F32, tag="p1")
        q = pool.tile([BPC, PAD], F32, tag="q")
        sh = pool.tile([BPC, NX + 1], BF16, tag="sh")
        m = pool.tile([BPC, NX], BF16, tag="m")
        bse = pool.tile([BPC, NX], F32, tag="bse")
        Pf = pool.tile([BPC, NX], F32, tag="Pf")
        alc = pool.tile([BPC, NX], F32, tag="alc")
        tmp = pool.tile([BPC, NX], F32, tag="tmp")
        rhs_v = pool.tile([2 * S, NX], F32R, tag="rhs_v")
        rhs_p = pool.tile([2 * S, NX], F32R, tag="rhs_p")
        a0 = pool.tile([2 * HID, NX], F32R, tag="a0")
        a1 = pool.tile([2 * HID, NX], F32R, tag="a1")
        a2 = pool.tile([2 * HID, NX], F32R, tag="a2")

        l0v_f = pool.tile([2 * S, 2 * HID], F32, tag="l0v_f")
        l0p_f = pool.tile([2 * S, 2 * HID], F32, tag="l0p_f")
        l1_f = pool.tile([2 * HID, 2 * HID], F32, tag="l1_f")
        l2_f = pool.tile([2 * HID, 2 * HID], F32, tag="l2_f")
        l3_f = pool.tile([2 * HID, BPC], F32, tag="l3_f")
        l0v = pool.tile([2 * S, 2 * HID], F32R, tag="l0v")
        l0p = pool.tile([2 * S, 2 * HID], F32R, tag="l0p")
        l1 = pool.tile([2 * HID, 2 * HID], F32R, tag="l1")
        l2 = pool.tile([2 * HID, 2 * HID], F32R, tag="l2")
        l3 = pool.tile([2 * HID, BPC], F32R, tag="l3")
        b0t = pool.tile([2 * HID, 1], F32, tag="b0t")
        b1t = pool.tile([2 * HID, 1], F32, tag="b1t")
        b2t = pool.tile([2 * HID, 1], F32, tag="b2t")
        b3t = pool.tile([BPC, 1], F32, tag="b3t")

        h0 = psum.tile([2 * HID, NX], F32, tag="h0")
        h1 = psum.tile([2 * HID, NX], F32, tag="h1")
        h2 = psum.tile([2 * HID, NX], F32, tag="h2")
        upd = psum.tile([BPC, NX], F32, tag="upd")

        # ---- one-time loads / inits ----
        nc.sync.dma_start(p0[:], d_state0[:])
        for t_, d_ in ((l0v_f, d_l0v), (l0p_f, d_l0p), (l1_f, d_l1),
                       (l2_f, d_l2), (l3_f, d_l3),
                       (b0t, d_b0), (b1t, d_b1), (b2t, d_b2), (b3t, d_b3)):
            nc.sync.dma_start(t_[:], d_[:])
        for src, dst in ((l0v_f, l0v), (l0p_f, l0p), (l1_f, l1),
                         (l2_f, l2), (l3_f, l3)):
            nc.vector.tensor_copy(dst[:], src[:])
        nc.vector.memset(alc[:], ALPHA)
        nc.vector.memset(sh[:, NX:NX + 1], 0.0)

        def sliding(ptile, c0, n):
            # [BPC, S, n] overlapping-window view starting at cell c0
            src = ptile[:, c0:c0 + S]
            fancy = src.copy()
            fancy.ap = src.ap + [[1, n]]
            return fancy.bitcast(F32R)

        def preops(p_cur, lo, hi):
            # sh[j] = state[j] > state[j-1] over [lo, hi); m/bse over [lo, hi)
            nc.vector.tensor_tensor(sh[:, lo:hi], p_cur[:, KHW + lo:KHW + hi],
                                    p_cur[:, KHW + lo - 1:KHW + hi - 1], OP.is_gt)
            nc.vector.tensor_tensor(m[:, lo:hi], sh[:, lo:hi],
                                    sh[:, lo + 1:hi + 1], OP.max)
            nc.vector.tensor_scalar(bse[:, lo:hi], m[:, lo:hi], BASE0, None,
                                    OP.mult)

        def ladder_half(c0, first):
            cs = slice(c0, c0 + CH)
            nc.tensor.matmul(h0[:, cs], l0v[:], rhs_v[:, cs],
                             start=True, stop=False)
            nc.tensor.matmul(h0[:, cs], l0p[:], rhs_p[:, cs],
                             start=False, stop=True)
            nc.scalar.activation(a0[:, cs], h0[:, cs], AF.Gelu, bias=b0t[:])
            nc.tensor.matmul(h1[:, cs], l1[:], a0[:, cs], start=True, stop=True)
            nc.scalar.activation(a1[:, cs], h1[:, cs], AF.Gelu, bias=b1t[:])
            nc.tensor.matmul(h2[:, cs], l2[:], a1[:, cs], start=True, stop=True)
            nc.scalar.activation(a2[:, cs], h2[:, cs], AF.Gelu, bias=b2t[:])
            nc.tensor.matmul(upd[:, cs], l3[:], a2[:, cs], start=True, stop=True)

        def finish_half(p_cur, c0):
            cs = slice(c0, c0 + CH)
            nc.vector.scalar_tensor_tensor(tmp[:, cs], upd[:, cs], b3t[:],
                                           p_prev[:, KHW + c0:KHW + c0 + CH],
                                           OP.add, OP.add)
            nc.vector.tensor_scalar(p_cur[:, KHW + c0:KHW + c0 + CH],
                                    tmp[:, cs], 0.0, 1.0, OP.max, OP.min)

        for t in range(NSTEP):
            p_prev = p0 if t % 2 == 0 else p1
            p_cur = p1 if t % 2 == 0 else p0

            # vals stencil DMAs (state fully ready from previous step)
            nc.scalar.dma_start(rhs_v[:], sliding(p_prev, 0, NX))

            if t == 0:
                # edges + full shock pre-ops for the initial state
                nc.vector.tensor_copy(p_prev[:, 0:KHW],
                                      p_prev[:, KHW:KHW + 1].broadcast_to([BPC, KHW]))
                nc.vector.tensor_copy(p_prev[:, KHW + NX:PAD],
                                      p_prev[:, KHW + NX - 1:KHW + NX]
                                      .broadcast_to([BPC, KHW]))
                preops(p_prev, 0, NX)

            # ---- distance transform in exp space (vector engine) ----
            nc.vector.tensor_tensor_scan(Pf[:], alc[:], bse[:], 0.0,
                                         OP.mult, OP.max)
            # backward scan, high part first (cells SPL..NX-1)
            nc.vector.tensor_tensor_scan(
                q[:, KHW + SPL:KHW + NX][:, ::-1], alc[:, 0:NX - SPL],
                Pf[:, SPL:NX][:, ::-1], 0.0, OP.mult, OP.max)
            nc.scalar.copy(q[:, KHW + NX:PAD],
                           q[:, KHW + NX - 1:KHW + NX].broadcast_to([BPC, KHW]))
            nc.sync.dma_start(rhs_p[:, CH:NX], sliding(q, CH, CH))
            # backward scan low part, chained via initial
            nc.vector.tensor_tensor_scan(
                q[:, KHW:KHW + SPL][:, ::-1], alc[:, 0:SPL],
                Pf[:, 0:SPL][:, ::-1], q[:, KHW + SPL:KHW + SPL + 1],
                OP.mult, OP.max)
            nc.scalar.copy(q[:, 0:KHW],
                           q[:, KHW:KHW + 1].broadcast_to([BPC, KHW]))
            nc.sync.dma_start(rhs_p[:, 0:CH], sliding(q, 0, CH))

            # ---- MLP ladder: high half first, then low half ----
            ladder_half(CH, True)
            ladder_half(0, False)

            # high half finishes first; its shock pre-ops run in the low
            # ladder's shadow
            finish_half(p_cur, CH)
            nc.vector.tensor_copy(p_cur[:, KHW + NX:PAD],
                                  p_cur[:, KHW + NX - 1:KHW + NX]
                                  .broadcast_to([BPC, KHW]))
            preops(p_cur, CH + 1, NX)

            finish_half(p_cur, 0)
            nc.vector.tensor_copy(p_cur[:, 0:KHW],
                                  p_cur[:, KHW:KHW + 1].broadcast_to([BPC, KHW]))
            preops(p_cur, 0, CH + 1)

            nc.scalar.dma_start(d_out[:, t, :], p_cur[:, KHW:KHW + NX])

    nc.compile()
    return nc


def _prepare_core_inputs(grid_input, dt, W0, b0, W1, b1, W2, b2, W3, b3):
    """Host-side constant folding; returns list of per-core input dicts."""
    f = np.float32
    W0 = W0.astype(f); W1 = W1.astype(f); W2 = W2.astype(f); W3 = W3.astype(f)
    W0v = W0[:, 0:S] - 2.0 * W0[:, S:2 * S]          # vals + folded char_speeds
    W0p = W0[:, 2 * S:3 * S]                          # prox columns

    def blockdiag(Wsub):  # Wsub [HID, k] -> [2k, 2HID]
        k = Wsub.shape[1]
        out = np.zeros((2 * k, 2 * HID), f)
        out[0:k, 0:HID] = Wsub.T
        out[k:2 * k, HID:2 * HID] = Wsub.T
        return out

    l0v = blockdiag(W0v)
    l0p = blockdiag(W0p)
    l1b = blockdiag(W1)
    l2b = blockdiag(W2)

    in_maps = []
    for c in range(NCORES):
        bsel = [BPC * c + i for i in range(BPC)]
        dts = dt[bsel].astype(f)
        s0 = grid_input[bsel, 0, 0, :].astype(f)
        s0p = np.concatenate([np.repeat(s0[:, :1], KHW, 1), s0,
                              np.repeat(s0[:, -1:], KHW, 1)], axis=1)
        b0d = np.concatenate([
            (b0 + W0[:, S:2 * S].sum(1) + W0[:, 3 * S] * dts[0]),
            (b0 + W0[:, S:2 * S].sum(1) + W0[:, 3 * S] * dts[1]),
        ]).astype(f)[:, None]
        b1d = np.concatenate([b1, b1]).astype(f)[:, None]
        b2d = np.concatenate([b2, b2]).astype(f)[:, None]
        l3 = np.zeros((2 * HID, BPC), f)
        l3[0:HID, 0] = W3[0] * (dts[0] / DX)
        l3[HID:2 * HID, 1] = W3[0] * (dts[1] / DX)
        b3d = np.array([[b3[0] * dts[0] / DX], [b3[0] * dts[1] / DX]], f)
        in_maps.append({
            "state0": s0p, "l0v": l0v, "l0p": l0p, "l1": l1b, "l2": l2b,
            "l3": l3, "b0d": b0d, "b1d": b1d, "b2d": b2d, "b3d": b3d,
        })
    return in_maps


def kernel(grid_input, dt, W0, b0, W1, b1, W2, b2, W3, b3, _run_kwargs=None):
    global _compiled
    grid_input = np.asarray(grid_input)
    if _compiled is None:
        _compiled = _build_module()
    nc = _compiled
    in_maps = _prepare_core_inputs(grid_input, np.asarray(dt),
                                   np.asarray(W0), np.asarray(b0),
                                   np.asarray(W1), np.asarray(b1),
                                   np.asarray(W2), np.asarray(b2),
                                   np.asarray(W3), np.asarray(b3))
    kw = _run_kwargs or {}
    r = run_bass_kernel_spmd(nc, in_maps, list(range(NCORES)), **kw)
    out = np.empty((B, 1, NT, NX), np.float32)
    out[:, 0, 0, :] = grid_input[:, 0, 0, :]
    for c in range(NCORES):
        out[BPC * c:BPC * (c + 1), 0, 1:, :] = r.results[c]["out"]
    kernel.last_results = r
    return out
